# revision 59
# baseline (speedup 1.0000x reference)
"""MoE forward (top-2 routed, 8 experts) on 8 Trainium2 NeuronCores.

Strategy: expert-parallel. The host computes the (cheap) routing decisions
and gathers each expert's assigned tokens (padded to capacity C); core e
computes the gating softmax on device (true fp32, so top-2 decisions match
the host routing) and its expert's FFN over its C tokens with fp32r matmuls
(full PE rate, ~1e-4 rounding) accumulated in fp32 PSUM.  Contributions
p * (ffn(x) + b2) are scattered into owner-core-grouped chunks and exchanged
with a single AllToAll; each core then gathers its own tokens' two expert
contributions, sums them, applies LayerNorm, and writes its 512-token shard.
The host concatenates the 8 shards.

Self-contained: hardcodes the problem shapes; compiles per routing capacity.
"""
import numpy as np

import concourse.bass as bass
import concourse.bacc as bacc
import concourse.tile as tile
import concourse.mybir as mybir
import concourse.bass_utils as bass_utils
from concourse.bass import IndirectOffsetOnAxis

P = 128
N_CORES = 8
TEMP = 0.9
EPS = 1e-5

f32 = mybir.dt.float32
f32r = mybir.dt.float32r
i32 = mybir.dt.int32
AF = mybir.ActivationFunctionType
OP = mybir.AluOpType


def _slices(C):
    """Token-slot slices for the compute/AllToAll pipeline (multiples of P).

    Single slice: a 2-slice pipeline was tried and the overlapped AllToAll's
    DMA traffic starved the PE of weight panels (net wash)."""
    return [(0, C)]


def _sub_blocks(n, pref=512):
    """Split n (multiple of 128) into matmul free-dim blocks <= 512,
    preferring >=256 (full-rate fp32r)."""
    out = []
    rem = n
    while rem > 0:
        if rem == 128 + pref:
            out += [384, 256]
            rem = 0
        elif rem >= pref:
            out.append(pref)
            rem -= pref
        else:
            out.append(rem)
            rem = 0
    return out


def build(C, C2s, D=1024, F=4096, E=8, T=4096):
    """Build the SPMD Bass program for capacity C (multiple of 128).

    C2s[s] = fixed per-(expert, owner-core) group capacity for slice s's
    AllToAll (one collective per compute slice, pipelined)."""
    DC, FC = D // P, F // P
    MG = FC // 4                  # m-groups of 4 F-chunks (w1/w2 panel unit)
    NCk = C // P                  # token chunks per core
    TOK = T // N_CORES            # tokens per output shard
    LNC = TOK // P                # LayerNorm tiles per shard
    assert D % P == 0 and F % (4 * P) == 0 and TOK % P == 0 and C % P == 0

    nc = bacc.Bacc("TRN2", target_bir_lowering=False, debug=False,
                   enable_asserts=True, num_devices=N_CORES)

    # All big inputs are host pre-tiled to [.., P, free] so each DMA is 128
    # contiguous per-partition descriptors (DMA queues are descriptor-bound
    # otherwise).
    xT = nc.dram_tensor("xT", [P, DC * C], f32, kind="ExternalInput").ap()
    xg_t = nc.dram_tensor("xg_t", [NCk, P, DC * P], f32,
                          kind="ExternalInput").ap()
    gw = nc.dram_tensor("gw", [D, E], f32, kind="ExternalInput").ap()
    gb = nc.dram_tensor("gb", [P, E], f32, kind="ExternalInput").ap()
    w1 = nc.dram_tensor("w1", [MG, P, DC * 4 * P], f32r,
                        kind="ExternalInput").ap()
    b1 = nc.dram_tensor("b1", [F], f32, kind="ExternalInput").ap()
    w2 = nc.dram_tensor("w2", [MG, P, 4 * D], f32r,
                        kind="ExternalInput").ap()
    b2 = nc.dram_tensor("b2", [P, D], f32, kind="ExternalInput").ap()
    # [q, ch, 2]: for shard token ch*P+q, the two source rows (e*C2 + pos)
    # in its half's post-AllToAll contribution tensor.
    gidx = nc.dram_tensor("gidx", [P, LNC * 2], i32, kind="ExternalInput").ap()
    # [slot]: destination row in the combined pre-AllToAll tensor (half h's
    # region starts at 8*sum(C2s[:h])); pad slots hold 2^31-1 and are dropped
    # by the scatter's bounds check.
    NH = len(C2s)
    dest2 = nc.dram_tensor("dest2", [C], i32, kind="ExternalInput").ap()
    lng = nc.dram_tensor("ln_g", [P, D], f32, kind="ExternalInput").ap()
    lnb = nc.dram_tensor("ln_b", [P, D], f32, kind="ExternalInput").ap()
    out = nc.dram_tensor("out", [TOK, D], f32, kind="ExternalOutput").ap()

    with tile.TileContext(nc) as tc:
        with (
            tc.tile_pool(name="res", bufs=1) as res,          # resident tiles
            tc.tile_pool(name="wpan", bufs=3) as wpan,        # weight panels
            tc.tile_pool(name="hbuf", bufs=1) as hbuf,        # h (gelu out)
            tc.tile_pool(name="work", bufs=2) as work,        # LN / misc
            tc.tile_pool(name="gat", bufs=1) as gat,          # gating temps
            tc.tile_pool(name="ps1", bufs=3, space="PSUM") as ps1p,
            tc.tile_pool(name="ps2", bufs=4, space="PSUM") as ps2p,
            tc.tile_pool(name="dram", bufs=1, space="DRAM") as dram,
        ):
            # ---- gating inputs first: PE can start while x/consts stream --
            gw_sb = res.tile([P, DC, E], f32)
            nc.sync.dma_start(gw_sb[:], gw.rearrange("(dc p) e -> p dc e", p=P))
            gb_sb = res.tile([P, E], f32)
            nc.sync.dma_start(gb_sb[:], gb[:, :])

            # ---------------- gating (fp32) ----------------
            logit = gat.tile([P, NCk, E], f32)
            for t in range(NCk):
                # True-fp32 load of this token chunk straight from DRAM: the
                # fp32r copy x_sb is rounded, and f32r rounding noise in the
                # gating logits can flip top-2 decisions vs the host routing.
                xg = work.tile([P, DC, P], f32, tag="xg")
                nc.sync.dma_start(
                    xg[:], xg_t[t].rearrange("p (dc q) -> p dc q", dc=DC))
                pg = ps1p.tile([P, E], f32, tag="ps1")
                for dc in range(DC):
                    nc.tensor.matmul(
                        pg[:],
                        xg[:, dc, :],
                        gw_sb[:, dc, :],
                        start=(dc == 0), stop=(dc == DC - 1),
                    )
                nc.vector.tensor_add(logit[:, t, :], pg[:], gb_sb[:])

            # ---------------- resident loads ----------------
            x_sb = res.tile([P, DC, C], f32r)
            # gpsimd cast DMAs: round fp32 -> fp32r once here (PE input prep);
            # split per D-chunk so the loads spread across DMA queues.
            for dc in range(DC):
                nc.gpsimd.dma_start(x_sb[:, dc, :], xT[:, dc * C:(dc + 1) * C])
            b1_sb = res.tile([P, FC], f32)
            nc.sync.dma_start(b1_sb[:], b1.rearrange("(fc p) -> p fc", p=P))
            b2_sb = res.tile([P, D], f32)
            nc.sync.dma_start(b2_sb[:], b2[:, :])
            lng_sb = res.tile([P, D], f32)
            nc.sync.dma_start(lng_sb[:], lng[:, :])
            lnb_sb = res.tile([P, D], f32)
            nc.sync.dma_start(lnb_sb[:], lnb[:, :])
            gidx_sb = res.tile([P, LNC, 2], i32)
            nc.sync.dma_start(gidx_sb[:],
                              gidx.rearrange("p (c k) -> p c k", k=2))
            dest2_sb = res.tile([P, NCk], i32)
            nc.sync.dma_start(dest2_sb[:], dest2.rearrange("(c p) -> p c", p=P))
            eps_sb = res.tile([P, 1], f32)
            nc.vector.memset(eps_sb[:], EPS)

            m1 = gat.tile([P, NCk, 1], f32)
            nc.vector.tensor_reduce(m1[:], logit[:], axis=mybir.AxisListType.X,
                                    op=OP.max)
            tt = gat.tile([P, NCk, E], f32)
            nc.vector.tensor_tensor(tt[:], logit[:],
                                    m1[:].to_broadcast([P, NCk, E]), OP.subtract)
            eq = gat.tile([P, NCk, E], f32)
            nc.vector.tensor_scalar(eq[:], tt[:], 0.0, None, OP.is_equal)
            msk = gat.tile([P, NCk, E], f32)
            nc.vector.scalar_tensor_tensor(msk[:], eq[:], -1e30, tt[:],
                                           OP.mult, OP.add)
            m2 = gat.tile([P, NCk, 1], f32)
            nc.vector.tensor_reduce(m2[:], msk[:], axis=mybir.AxisListType.X,
                                    op=OP.max)
            keep = gat.tile([P, NCk, E], f32)
            nc.vector.tensor_tensor(keep[:], tt[:],
                                    m2[:].to_broadcast([P, NCk, E]), OP.is_ge)
            ex = gat.tile([P, NCk, E], f32)
            nc.scalar.activation(ex[:], tt[:], AF.Exp, scale=float(1.0 / TEMP))
            exk = gat.tile([P, NCk, E], f32)
            nc.vector.tensor_mul(exk[:], ex[:], keep[:])
            ssum = gat.tile([P, NCk, 1], f32)
            nc.vector.tensor_reduce(ssum[:], exk[:], axis=mybir.AxisListType.X,
                                    op=OP.add)
            rcp = gat.tile([P, NCk, 1], f32)
            nc.vector.reciprocal(rcp[:], ssum[:])
            pcols = gat.tile([P, NCk], f32)
            nc.vector.tensor_tensor(pcols[:], exk[:, :, 0],
                                    rcp[:, :, 0], OP.mult)

            # ---------------- y accumulator init: y = p * b2 --------------
            y_acc = res.tile([P, NCk, D], f32)
            for mt in range(NCk):
                nc.vector.tensor_scalar(y_acc[:, mt, :], b2_sb[:],
                                        pcols[:, mt:mt + 1], None, OP.mult)

            # ---------------- main FFN loop ----------------
            ndsubs = _sub_blocks(D)
            for sc0, scn in _slices(C):
                for mg in range(MG):
                    w1p = wpan.tile([P, DC, 4 * P], f32r, tag="wp")
                    nc.sync.dma_start(
                        w1p[:], w1[mg].rearrange("p (dc m) -> p dc m", dc=DC))
                    w2p = wpan.tile([P, 4, D], f32r, tag="wp")
                    nc.sync.dma_start(
                        w2p[:], w2[mg].rearrange("p (mi d) -> p mi d", mi=4))
                    h_mg = hbuf.tile([P, 4, scn], f32r, tag="h")
                    # mm1: h = gelu(x @ w1 + b1), transposed [F-part, tokens]
                    for mi in range(4):
                        c0 = 0
                        for ns in _sub_blocks(scn):
                            ps = ps1p.tile([P, ns], f32, tag="ps1")
                            for dc in range(DC):
                                nc.tensor.matmul(
                                    ps[:],
                                    w1p[:, dc, mi * P:(mi + 1) * P],
                                    x_sb[:, dc, sc0 + c0:sc0 + c0 + ns],
                                    start=(dc == 0), stop=(dc == DC - 1),
                                )
                            nc.scalar.activation(
                                h_mg[:, mi, c0:c0 + ns], ps[:], AF.Gelu,
                                bias=b1_sb[:, mg * 4 + mi:mg * 4 + mi + 1])
                            c0 += ns
                    # mm2: y[tok, d] += p * (h.T @ w2)
                    for mtl in range(scn // P):
                        mt = sc0 // P + mtl
                        d0 = 0
                        for nd in ndsubs:
                            ps = ps2p.tile([P, nd], f32, tag="ps2")
                            for mi in range(4):
                                nc.tensor.matmul(
                                    ps[:],
                                    h_mg[:, mi, mtl * P:(mtl + 1) * P],
                                    w2p[:, mi, d0:d0 + nd],
                                    start=(mi == 0), stop=(mi == 3),
                                )
                            nc.vector.scalar_tensor_tensor(
                                y_acc[:, mt, d0:d0 + nd], ps[:],
                                pcols[:, mt:mt + 1], y_acc[:, mt, d0:d0 + nd],
                                OP.mult, OP.add)
                            d0 += nd

            # ---- ship contributions in NH owner-position halves: the LN of
            # half h overlaps half h+1's AllToAll transfer.  One scatter per
            # chunk into a combined tensor; each half's region feeds its own
            # collective.
            ya_off = [0]
            for h in range(NH):
                ya_off.append(ya_off[-1] + N_CORES * C2s[h])
            y_all = dram.tile([ya_off[-1], D], f32)
            for mt in range(NCk):
                nc.gpsimd.indirect_dma_start(
                    out=y_all[:, :],
                    out_offset=IndirectOffsetOnAxis(
                        ap=dest2_sb[:, mt:mt + 1], axis=0),
                    in_=y_acc[:, mt, :],
                    in_offset=None,
                    bounds_check=ya_off[-1] - 1,
                    oob_is_err=False,
                )
            ags = []
            for h in range(NH):
                ag = dram.tile([N_CORES * C2s[h], D], f32, tag=f"ag{h}",
                               name=f"ag{h}")
                nc.gpsimd.collective_compute(
                    "AllToAll",
                    OP.bypass,
                    replica_groups=[list(range(N_CORES))],
                    ins=[y_all[ya_off[h]:ya_off[h + 1], :].opt()],
                    outs=[ag.opt()],
                )
                ags.append(ag)

            # -------- per-shard combine (2 contributions) + LayerNorm ------
            for ch in range(LNC):
                ag = ags[ch * NH // LNC]
                xt = work.tile([P, D], f32, tag="ln_x")
                nc.gpsimd.indirect_dma_start(
                    out=xt[:], out_offset=None,
                    in_=ag[:, :],
                    in_offset=IndirectOffsetOnAxis(
                        ap=gidx_sb[:, ch, 0:1], axis=0),
                )
                # second gather accumulates in place (DMA compute-op add)
                nc.gpsimd.indirect_dma_start(
                    out=xt[:], out_offset=None,
                    in_=ag[:, :],
                    in_offset=IndirectOffsetOnAxis(
                        ap=gidx_sb[:, ch, 1:2], axis=0),
                    compute_op=OP.add,
                )
                s1 = work.tile([P, 1], f32, tag="ln_s1")
                nc.vector.tensor_reduce(s1[:], xt[:], axis=mybir.AxisListType.X,
                                        op=OP.add)
                scr = work.tile([P, D], f32, tag="ln_scr")
                s2 = work.tile([P, 1], f32, tag="ln_s2")
                nc.scalar.activation(scr[:], xt[:], AF.Square, accum_out=s2[:])
                mean = work.tile([P, 1], f32, tag="ln_mean")
                nc.vector.tensor_scalar_mul(mean[:], s1[:], 1.0 / D)
                ex2 = work.tile([P, 1], f32, tag="ln_ex2")
                nc.vector.tensor_scalar_mul(ex2[:], s2[:], 1.0 / D)
                # var = ex2 - mean^2  (computed as (-mean)*mean + ex2)
                var = work.tile([P, 1], f32, tag="ln_var")
                nc.vector.scalar_tensor_tensor(var[:], mean[:], -1.0, mean[:],
                                               OP.mult, OP.mult)
                nc.vector.tensor_add(var[:], var[:], ex2[:])
                sd = work.tile([P, 1], f32, tag="ln_sd")
                nc.scalar.activation(sd[:], var[:], AF.Sqrt, bias=eps_sb[:])
                rstd = work.tile([P, 1], f32, tag="ln_rstd")
                nc.vector.reciprocal(rstd[:], sd[:])
                nrm = work.tile([P, D], f32, tag="ln_nrm")
                nc.vector.tensor_scalar(nrm[:], xt[:], mean[:], rstd[:],
                                        OP.subtract, OP.mult)
                nc.vector.tensor_mul(nrm[:], nrm[:], lng_sb[:])
                nc.vector.tensor_add(nrm[:], nrm[:], lnb_sb[:])
                nc.sync.dma_start(out[ch * P:(ch + 1) * P, :], nrm[:])

    nc.compile()
    return nc


_CACHE = {}


def _get_program(C, C2s, D, F, E, T):
    key = (C, tuple(C2s), D, F, E, T)
    if key not in _CACHE:
        _CACHE[key] = build(C, tuple(C2s), D=D, F=F, E=E, T=T)
    return _CACHE[key]


def prepare(x, gate_w, gate_b, w1, b1, w2, b2, ln_g, ln_b):
    """Host-side routing + per-core input construction.

    Returns (C, dims, in_maps)."""
    x = np.asarray(x, np.float32)
    gate_w = np.asarray(gate_w, np.float32)
    gate_b = np.asarray(gate_b, np.float32)
    w1 = np.asarray(w1, np.float32)
    b1 = np.asarray(b1, np.float32)
    w2 = np.asarray(w2, np.float32)
    b2 = np.asarray(b2, np.float32)
    ln_g = np.asarray(ln_g, np.float32)
    ln_b = np.asarray(ln_b, np.float32)

    Bb, S, D = x.shape
    E, _, F = w1.shape
    T = Bb * S
    TOK = T // N_CORES

    # ---- host routing (decisions only; probabilities are computed on device)
    x2 = np.ascontiguousarray(x.reshape(T, D))
    logits = (x2 @ gate_w + gate_b) / TEMP
    top2 = np.argsort(-logits, axis=1, kind="stable")[:, :2]
    idx = [np.where((top2 == e).any(axis=1))[0] for e in range(E)]
    maxc = max(len(i) for i in idx)
    C = max(((maxc + P - 1) // P) * P, P)

    DC, NCk, MG = D // P, C // P, F // (4 * P)
    LNC = TOK // P

    # Owner-(core, shard-half) grouping for the AllToAlls: half h carries the
    # contributions of shard rows [h*TOK/NH, (h+1)*TOK/NH) of every core, so
    # half 0's LayerNorm overlaps half 1's transfer.  Within an expert batch
    # (ids ascending) each (owner, half) group is a contiguous run.
    NH = 2 if LNC >= 2 else 1
    HTOK = TOK // NH
    cnt_h = np.zeros((NH, E, N_CORES), np.int64)
    for e in range(E):
        ids = idx[e]
        owner = ids // TOK
        half = (ids % TOK) // HTOK
        for h in range(NH):
            cnt_h[h, e] = np.bincount(owner[half == h], minlength=N_CORES)
    C2s = [max(16, int(((cnt_h[h].max() + 15) // 16) * 16))
           for h in range(NH)]

    # rows[t, k]: source row of (token t, choice k) in its half's
    # post-AllToAll tensor on t's owner core.  Pads / other-half slots get
    # BIG and are dropped by the scatter bounds check.
    BIG = np.int32(2**31 - 1)
    ya_off = np.concatenate(
        [[0], np.cumsum([N_CORES * c2 for c2 in C2s])]).astype(np.int64)
    rows = np.empty((T, 2), np.int32)
    dest2s = [np.full((C,), BIG, np.int32) for _ in range(E)]
    for e in range(E):
        ids = idx[e]
        n = len(ids)
        owner = ids // TOK
        half = (ids % TOK) // HTOK
        key = half * N_CORES + owner
        kcnt = np.bincount(key, minlength=NH * N_CORES)
        gstart = np.concatenate([[0], np.cumsum(kcnt)[:-1]])
        order = np.argsort(key, kind="stable")
        pos = np.empty(n, np.int64)
        pos[order] = np.arange(n) - np.repeat(gstart, kcnt)
        dest2s[e][:n] = (ya_off[half] + owner * np.take(C2s, half)
                         + pos).astype(np.int32)
        for k in (0, 1):
            mk = top2[ids, k] == e
            rows[ids[mk], k] = (
                e * np.take(C2s, half[mk]) + pos[mk]).astype(np.int32)

    in_maps = []
    for e in range(E):
        ids = idx[e]
        xTe = np.zeros((D, C), np.float32)
        xTe[:, :len(ids)] = x2[ids].T
        xT_t = np.ascontiguousarray(
            xTe.reshape(DC, P, C).transpose(1, 0, 2).reshape(P, DC * C))
        xg_t = np.ascontiguousarray(
            xTe.reshape(DC, P, NCk, P).transpose(2, 1, 0, 3)
            .reshape(NCk, P, DC * P))
        w1t = np.ascontiguousarray(
            w1[e].reshape(DC, P, MG, 4 * P).transpose(2, 1, 0, 3)
            .reshape(MG, P, DC * 4 * P))
        w2t = np.ascontiguousarray(
            w2[e].reshape(MG, 4, P, D).transpose(0, 2, 1, 3)
            .reshape(MG, P, 4 * D))
        gidx_core = np.ascontiguousarray(
            rows[e * TOK:(e + 1) * TOK].reshape(LNC, P, 2)
            .transpose(1, 0, 2).reshape(P, LNC * 2))
        perm = np.roll(np.arange(E), -e)   # col 0 = own expert
        in_maps.append({
            "xT": xT_t,
            "xg_t": xg_t,
            "gw": np.ascontiguousarray(gate_w[:, perm]),
            "gb": np.ascontiguousarray(
                np.broadcast_to(gate_b[perm], (P, E))),
            "w1": w1t,
            "b1": np.ascontiguousarray(b1[e]),
            "w2": w2t,
            "b2": np.ascontiguousarray(np.broadcast_to(b2[e], (P, D))),
            "gidx": gidx_core,
            "dest2": dest2s[e],
            "ln_g": np.ascontiguousarray(np.broadcast_to(ln_g, (P, D))),
            "ln_b": np.ascontiguousarray(np.broadcast_to(ln_b, (P, D))),
        })

    return C, tuple(C2s), (Bb, S, D, F, E, T), in_maps


def kernel(x, gate_w, gate_b, w1, b1, w2, b2, ln_g, ln_b):
    C, C2s, (Bb, S, D, F, E, T), in_maps = prepare(
        x, gate_w, gate_b, w1, b1, w2, b2, ln_g, ln_b)
    nc = _get_program(C, C2s, D, F, E, T)
    res = bass_utils.run_bass_kernel_spmd(
        nc, in_maps, core_ids=list(range(N_CORES)))
    shards = [res.results[c]["out"] for c in range(N_CORES)]
    return np.concatenate(shards, axis=0).reshape(Bb, S, D)


# revision 60
# speedup vs baseline: 1.0139x; 1.0139x over previous
"""MoE forward (top-2 routed, 8 experts) on 8 Trainium2 NeuronCores.

Strategy: expert-parallel. The host computes the (cheap) routing decisions
and gathers each expert's assigned tokens (padded to capacity C); core e
computes the gating softmax on device (true fp32, so top-2 decisions match
the host routing) and its expert's FFN over its C tokens with fp32r matmuls
(full PE rate, ~1e-4 rounding) accumulated in fp32 PSUM.  Contributions
p * (ffn(x) + b2) are scattered into owner-core-grouped chunks and exchanged
with a single AllToAll; each core then gathers its own tokens' two expert
contributions, sums them, applies LayerNorm, and writes its 512-token shard.
The host concatenates the 8 shards.

Self-contained: hardcodes the problem shapes; compiles per routing capacity.
"""
import numpy as np

import concourse.bass as bass
import concourse.bacc as bacc
import concourse.tile as tile
import concourse.mybir as mybir
import concourse.bass_utils as bass_utils
from concourse.bass import IndirectOffsetOnAxis

P = 128
N_CORES = 8
TEMP = 0.9
EPS = 1e-5

f32 = mybir.dt.float32
f32r = mybir.dt.float32r
i32 = mybir.dt.int32
AF = mybir.ActivationFunctionType
OP = mybir.AluOpType


def _slices(C):
    """Token-slot slices for the compute/AllToAll pipeline (multiples of P).

    Single slice: a 2-slice pipeline was tried and the overlapped AllToAll's
    DMA traffic starved the PE of weight panels (net wash)."""
    return [(0, C)]


def _sub_blocks(n, pref=512):
    """Split n (multiple of 128) into matmul free-dim blocks <= 512,
    preferring >=256 (full-rate fp32r)."""
    out = []
    rem = n
    while rem > 0:
        if rem == 128 + pref:
            out += [384, 256]
            rem = 0
        elif rem >= pref:
            out.append(pref)
            rem -= pref
        else:
            out.append(rem)
            rem = 0
    return out


def build(C, C2s, D=1024, F=4096, E=8, T=4096):
    """Build the SPMD Bass program for capacity C (multiple of 128).

    C2s[s] = fixed per-(expert, owner-core) group capacity for slice s's
    AllToAll (one collective per compute slice, pipelined)."""
    DC, FC = D // P, F // P
    MG = FC // 4                  # m-groups of 4 F-chunks (w1/w2 panel unit)
    NCk = C // P                  # token chunks per core
    TOK = T // N_CORES            # tokens per output shard
    LNC = TOK // P                # LayerNorm tiles per shard
    assert D % P == 0 and F % (4 * P) == 0 and TOK % P == 0 and C % P == 0

    nc = bacc.Bacc("TRN2", target_bir_lowering=False, debug=False,
                   enable_asserts=True, num_devices=N_CORES)

    # All big inputs are host pre-tiled to [.., P, free] so each DMA is 128
    # contiguous per-partition descriptors (DMA queues are descriptor-bound
    # otherwise).
    xT = nc.dram_tensor("xT", [P, DC * C], f32, kind="ExternalInput").ap()
    xg_t = nc.dram_tensor("xg_t", [NCk, P, DC * P], f32,
                          kind="ExternalInput").ap()
    gw = nc.dram_tensor("gw", [D, E], f32, kind="ExternalInput").ap()
    gb = nc.dram_tensor("gb", [P, E], f32, kind="ExternalInput").ap()
    w1 = nc.dram_tensor("w1", [MG, P, DC * 4 * P], f32r,
                        kind="ExternalInput").ap()
    b1 = nc.dram_tensor("b1", [F], f32, kind="ExternalInput").ap()
    w2 = nc.dram_tensor("w2", [MG, P, 4 * D], f32r,
                        kind="ExternalInput").ap()
    b2 = nc.dram_tensor("b2", [P, D], f32, kind="ExternalInput").ap()
    # [q, ch, 2]: for shard token ch*P+q, the two source rows (e*C2 + pos)
    # in its half's post-AllToAll contribution tensor.
    gidx = nc.dram_tensor("gidx", [P, LNC * 2], i32, kind="ExternalInput").ap()
    # [slot]: destination row in the combined pre-AllToAll tensor (half h's
    # region starts at 8*sum(C2s[:h])); pad slots hold 2^31-1 and are dropped
    # by the scatter's bounds check.
    NH = len(C2s)
    dest2 = nc.dram_tensor("dest2", [C], i32, kind="ExternalInput").ap()
    lng = nc.dram_tensor("ln_g", [P, D], f32, kind="ExternalInput").ap()
    lnb = nc.dram_tensor("ln_b", [P, D], f32, kind="ExternalInput").ap()
    out = nc.dram_tensor("out", [TOK, D], f32, kind="ExternalOutput").ap()

    with tile.TileContext(nc) as tc:
        with (
            tc.tile_pool(name="res", bufs=1) as res,          # resident tiles
            tc.tile_pool(name="wpan", bufs=3) as wpan,        # weight panels
            tc.tile_pool(name="hbuf", bufs=1) as hbuf,        # h (gelu out)
            tc.tile_pool(name="work", bufs=2) as work,        # LN / misc
            tc.tile_pool(name="gat", bufs=1) as gat,          # gating temps
            tc.tile_pool(name="ps1", bufs=3, space="PSUM") as ps1p,
            tc.tile_pool(name="ps2", bufs=4, space="PSUM") as ps2p,
            tc.tile_pool(name="dram", bufs=1, space="DRAM") as dram,
        ):
            # ---- gating inputs first: PE can start while x/consts stream --
            gw_sb = res.tile([P, DC, E], f32)
            nc.sync.dma_start(gw_sb[:], gw.rearrange("(dc p) e -> p dc e", p=P))
            gb_sb = res.tile([P, E], f32)
            nc.sync.dma_start(gb_sb[:], gb[:, :])

            # ---------------- gating (fp32) ----------------
            logit = gat.tile([P, NCk, E], f32)
            for t in range(NCk):
                # True-fp32 load of this token chunk straight from DRAM: the
                # fp32r copy x_sb is rounded, and f32r rounding noise in the
                # gating logits can flip top-2 decisions vs the host routing.
                xg = work.tile([P, DC, P], f32, tag="xg")
                nc.sync.dma_start(
                    xg[:], xg_t[t].rearrange("p (dc q) -> p dc q", dc=DC))
                pg = ps1p.tile([P, E], f32, tag="ps1")
                for dc in range(DC):
                    nc.tensor.matmul(
                        pg[:],
                        xg[:, dc, :],
                        gw_sb[:, dc, :],
                        start=(dc == 0), stop=(dc == DC - 1),
                    )
                nc.vector.tensor_add(logit[:, t, :], pg[:], gb_sb[:])

            # ---------------- resident loads ----------------
            x_sb = res.tile([P, DC, C], f32r)
            # gpsimd cast DMAs: round fp32 -> fp32r once here (PE input prep);
            # split per D-chunk so the loads spread across DMA queues.
            for dc in range(DC):
                nc.gpsimd.dma_start(x_sb[:, dc, :], xT[:, dc * C:(dc + 1) * C])
            b1_sb = res.tile([P, FC], f32)
            nc.sync.dma_start(b1_sb[:], b1.rearrange("(fc p) -> p fc", p=P))
            b2_sb = res.tile([P, D], f32)
            nc.sync.dma_start(b2_sb[:], b2[:, :])
            lng_sb = res.tile([P, D], f32)
            nc.sync.dma_start(lng_sb[:], lng[:, :])
            lnb_sb = res.tile([P, D], f32)
            nc.sync.dma_start(lnb_sb[:], lnb[:, :])
            gidx_sb = res.tile([P, LNC, 2], i32)
            nc.sync.dma_start(gidx_sb[:],
                              gidx.rearrange("p (c k) -> p c k", k=2))
            dest2_sb = res.tile([P, NCk], i32)
            nc.sync.dma_start(dest2_sb[:], dest2.rearrange("(c p) -> p c", p=P))
            eps_sb = res.tile([P, 1], f32)
            nc.vector.memset(eps_sb[:], EPS)

            m1 = gat.tile([P, NCk, 1], f32)
            nc.vector.tensor_reduce(m1[:], logit[:], axis=mybir.AxisListType.X,
                                    op=OP.max)
            tt = gat.tile([P, NCk, E], f32)
            nc.vector.tensor_tensor(tt[:], logit[:],
                                    m1[:].to_broadcast([P, NCk, E]), OP.subtract)
            eq = gat.tile([P, NCk, E], f32)
            nc.vector.tensor_scalar(eq[:], tt[:], 0.0, None, OP.is_equal)
            msk = gat.tile([P, NCk, E], f32)
            nc.vector.scalar_tensor_tensor(msk[:], eq[:], -1e30, tt[:],
                                           OP.mult, OP.add)
            m2 = gat.tile([P, NCk, 1], f32)
            nc.vector.tensor_reduce(m2[:], msk[:], axis=mybir.AxisListType.X,
                                    op=OP.max)
            keep = gat.tile([P, NCk, E], f32)
            nc.vector.tensor_tensor(keep[:], tt[:],
                                    m2[:].to_broadcast([P, NCk, E]), OP.is_ge)
            ex = gat.tile([P, NCk, E], f32)
            nc.scalar.activation(ex[:], tt[:], AF.Exp, scale=float(1.0 / TEMP))
            exk = gat.tile([P, NCk, E], f32)
            nc.vector.tensor_mul(exk[:], ex[:], keep[:])
            ssum = gat.tile([P, NCk, 1], f32)
            nc.vector.tensor_reduce(ssum[:], exk[:], axis=mybir.AxisListType.X,
                                    op=OP.add)
            rcp = gat.tile([P, NCk, 1], f32)
            nc.vector.reciprocal(rcp[:], ssum[:])
            pcols = gat.tile([P, NCk], f32)
            nc.vector.tensor_tensor(pcols[:], exk[:, :, 0],
                                    rcp[:, :, 0], OP.mult)

            # ---------------- y accumulator init: y = p * b2 --------------
            y_acc = res.tile([P, NCk, D], f32)
            for mt in range(NCk):
                nc.vector.tensor_scalar(y_acc[:, mt, :], b2_sb[:],
                                        pcols[:, mt:mt + 1], None, OP.mult)

            # ---------------- main FFN loop ----------------
            ndsubs = _sub_blocks(D)
            for sc0, scn in _slices(C):
                for mg in range(MG):
                    w1p = wpan.tile([P, DC, 4 * P], f32r, tag="wp")
                    nc.sync.dma_start(
                        w1p[:], w1[mg].rearrange("p (dc m) -> p dc m", dc=DC))
                    w2p = wpan.tile([P, 4, D], f32r, tag="wp")
                    nc.sync.dma_start(
                        w2p[:], w2[mg].rearrange("p (mi d) -> p mi d", mi=4))
                    h_mg = hbuf.tile([P, 4, scn], f32r, tag="h")
                    # mm1: h = gelu(x @ w1 + b1), transposed [F-part, tokens]
                    for mi in range(4):
                        c0 = 0
                        for ns in _sub_blocks(scn):
                            ps = ps1p.tile([P, ns], f32, tag="ps1")
                            for dc in range(DC):
                                nc.tensor.matmul(
                                    ps[:],
                                    w1p[:, dc, mi * P:(mi + 1) * P],
                                    x_sb[:, dc, sc0 + c0:sc0 + c0 + ns],
                                    start=(dc == 0), stop=(dc == DC - 1),
                                )
                            nc.scalar.activation(
                                h_mg[:, mi, c0:c0 + ns], ps[:], AF.Gelu,
                                bias=b1_sb[:, mg * 4 + mi:mg * 4 + mi + 1])
                            c0 += ns
                    # mm2: y[tok, d] += p * (h.T @ w2)
                    for mtl in range(scn // P):
                        mt = sc0 // P + mtl
                        d0 = 0
                        for nd in ndsubs:
                            ps = ps2p.tile([P, nd], f32, tag="ps2")
                            for mi in range(4):
                                nc.tensor.matmul(
                                    ps[:],
                                    h_mg[:, mi, mtl * P:(mtl + 1) * P],
                                    w2p[:, mi, d0:d0 + nd],
                                    start=(mi == 0), stop=(mi == 3),
                                )
                            nc.vector.scalar_tensor_tensor(
                                y_acc[:, mt, d0:d0 + nd], ps[:],
                                pcols[:, mt:mt + 1], y_acc[:, mt, d0:d0 + nd],
                                OP.mult, OP.add)
                            d0 += nd

            # ---- ship contributions in NH owner-position halves: the LN of
            # half h overlaps half h+1's AllToAll transfer.  One scatter per
            # chunk into a combined tensor; each half's region feeds its own
            # collective.
            ya_off = [0]
            for h in range(NH):
                ya_off.append(ya_off[-1] + N_CORES * C2s[h])
            y_all = dram.tile([ya_off[-1], D], f32)
            for mt in range(NCk):
                nc.gpsimd.indirect_dma_start(
                    out=y_all[:, :],
                    out_offset=IndirectOffsetOnAxis(
                        ap=dest2_sb[:, mt:mt + 1], axis=0),
                    in_=y_acc[:, mt, :],
                    in_offset=None,
                    bounds_check=ya_off[-1] - 1,
                    oob_is_err=False,
                )
            ags = []
            for h in range(NH):
                ag = dram.tile([N_CORES * C2s[h], D], f32, tag=f"ag{h}",
                               name=f"ag{h}")
                nc.gpsimd.collective_compute(
                    "AllToAll",
                    OP.bypass,
                    replica_groups=[list(range(N_CORES))],
                    ins=[y_all[ya_off[h]:ya_off[h + 1], :].opt()],
                    outs=[ag.opt()],
                )
                ags.append(ag)

            # -------- per-shard combine (2 contributions) + LayerNorm ------
            for ch in range(LNC):
                ag = ags[ch * NH // LNC]
                xt = work.tile([P, D], f32, tag="ln_x")
                nc.gpsimd.indirect_dma_start(
                    out=xt[:], out_offset=None,
                    in_=ag[:, :],
                    in_offset=IndirectOffsetOnAxis(
                        ap=gidx_sb[:, ch, 0:1], axis=0),
                )
                # second gather accumulates in place (DMA compute-op add)
                nc.gpsimd.indirect_dma_start(
                    out=xt[:], out_offset=None,
                    in_=ag[:, :],
                    in_offset=IndirectOffsetOnAxis(
                        ap=gidx_sb[:, ch, 1:2], axis=0),
                    compute_op=OP.add,
                )
                s1 = work.tile([P, 1], f32, tag="ln_s1")
                nc.vector.tensor_reduce(s1[:], xt[:], axis=mybir.AxisListType.X,
                                        op=OP.add)
                scr = work.tile([P, D], f32, tag="ln_scr")
                s2 = work.tile([P, 1], f32, tag="ln_s2")
                nc.scalar.activation(scr[:], xt[:], AF.Square, accum_out=s2[:])
                mean = work.tile([P, 1], f32, tag="ln_mean")
                nc.vector.tensor_scalar_mul(mean[:], s1[:], 1.0 / D)
                ex2 = work.tile([P, 1], f32, tag="ln_ex2")
                nc.vector.tensor_scalar_mul(ex2[:], s2[:], 1.0 / D)
                # var = ex2 - mean^2  (computed as (-mean)*mean + ex2)
                var = work.tile([P, 1], f32, tag="ln_var")
                nc.vector.scalar_tensor_tensor(var[:], mean[:], -1.0, mean[:],
                                               OP.mult, OP.mult)
                nc.vector.tensor_add(var[:], var[:], ex2[:])
                sd = work.tile([P, 1], f32, tag="ln_sd")
                nc.scalar.activation(sd[:], var[:], AF.Sqrt, bias=eps_sb[:])
                rstd = work.tile([P, 1], f32, tag="ln_rstd")
                nc.vector.reciprocal(rstd[:], sd[:])
                nrm = work.tile([P, D], f32, tag="ln_nrm")
                nc.vector.tensor_scalar(nrm[:], xt[:], mean[:], rstd[:],
                                        OP.subtract, OP.mult)
                nc.vector.tensor_mul(nrm[:], nrm[:], lng_sb[:])
                nc.vector.tensor_add(nrm[:], nrm[:], lnb_sb[:])
                nc.sync.dma_start(out[ch * P:(ch + 1) * P, :], nrm[:])

    nc.compile()
    return nc


_CACHE = {}


def _get_program(C, C2s, D, F, E, T):
    key = (C, tuple(C2s), D, F, E, T)
    if key not in _CACHE:
        _CACHE[key] = build(C, tuple(C2s), D=D, F=F, E=E, T=T)
    return _CACHE[key]


def prepare(x, gate_w, gate_b, w1, b1, w2, b2, ln_g, ln_b):
    """Host-side routing + per-core input construction.

    Returns (C, dims, in_maps)."""
    x = np.asarray(x, np.float32)
    gate_w = np.asarray(gate_w, np.float32)
    gate_b = np.asarray(gate_b, np.float32)
    w1 = np.asarray(w1, np.float32)
    b1 = np.asarray(b1, np.float32)
    w2 = np.asarray(w2, np.float32)
    b2 = np.asarray(b2, np.float32)
    ln_g = np.asarray(ln_g, np.float32)
    ln_b = np.asarray(ln_b, np.float32)

    Bb, S, D = x.shape
    E, _, F = w1.shape
    T = Bb * S
    TOK = T // N_CORES

    # ---- host routing (decisions only; probabilities are computed on device)
    x2 = np.ascontiguousarray(x.reshape(T, D))
    logits = (x2 @ gate_w + gate_b) / TEMP
    top2 = np.argsort(-logits, axis=1, kind="stable")[:, :2]
    idx = [np.where((top2 == e).any(axis=1))[0] for e in range(E)]
    maxc = max(len(i) for i in idx)
    C = max(((maxc + P - 1) // P) * P, P)

    DC, NCk, MG = D // P, C // P, F // (4 * P)
    LNC = TOK // P

    # Owner-(core, shard-half) grouping for the AllToAlls: half h carries the
    # contributions of shard rows [h*TOK/NH, (h+1)*TOK/NH) of every core, so
    # half 0's LayerNorm overlaps half 1's transfer.  Within an expert batch
    # (ids ascending) each (owner, half) group is a contiguous run.
    NH = 1  # half-split A2A measured as a wash vs single collective
    HTOK = TOK // NH
    cnt_h = np.zeros((NH, E, N_CORES), np.int64)
    for e in range(E):
        ids = idx[e]
        owner = ids // TOK
        half = (ids % TOK) // HTOK
        for h in range(NH):
            cnt_h[h, e] = np.bincount(owner[half == h], minlength=N_CORES)
    C2s = [max(16, int(((cnt_h[h].max() + 15) // 16) * 16))
           for h in range(NH)]

    # rows[t, k]: source row of (token t, choice k) in its half's
    # post-AllToAll tensor on t's owner core.  Pads / other-half slots get
    # BIG and are dropped by the scatter bounds check.
    BIG = np.int32(2**31 - 1)
    ya_off = np.concatenate(
        [[0], np.cumsum([N_CORES * c2 for c2 in C2s])]).astype(np.int64)
    rows = np.empty((T, 2), np.int32)
    dest2s = [np.full((C,), BIG, np.int32) for _ in range(E)]
    for e in range(E):
        ids = idx[e]
        n = len(ids)
        owner = ids // TOK
        half = (ids % TOK) // HTOK
        key = half * N_CORES + owner
        kcnt = np.bincount(key, minlength=NH * N_CORES)
        gstart = np.concatenate([[0], np.cumsum(kcnt)[:-1]])
        order = np.argsort(key, kind="stable")
        pos = np.empty(n, np.int64)
        pos[order] = np.arange(n) - np.repeat(gstart, kcnt)
        dest2s[e][:n] = (ya_off[half] + owner * np.take(C2s, half)
                         + pos).astype(np.int32)
        for k in (0, 1):
            mk = top2[ids, k] == e
            rows[ids[mk], k] = (
                e * np.take(C2s, half[mk]) + pos[mk]).astype(np.int32)

    in_maps = []
    for e in range(E):
        ids = idx[e]
        xTe = np.zeros((D, C), np.float32)
        xTe[:, :len(ids)] = x2[ids].T
        xT_t = np.ascontiguousarray(
            xTe.reshape(DC, P, C).transpose(1, 0, 2).reshape(P, DC * C))
        xg_t = np.ascontiguousarray(
            xTe.reshape(DC, P, NCk, P).transpose(2, 1, 0, 3)
            .reshape(NCk, P, DC * P))
        w1t = np.ascontiguousarray(
            w1[e].reshape(DC, P, MG, 4 * P).transpose(2, 1, 0, 3)
            .reshape(MG, P, DC * 4 * P))
        w2t = np.ascontiguousarray(
            w2[e].reshape(MG, 4, P, D).transpose(0, 2, 1, 3)
            .reshape(MG, P, 4 * D))
        gidx_core = np.ascontiguousarray(
            rows[e * TOK:(e + 1) * TOK].reshape(LNC, P, 2)
            .transpose(1, 0, 2).reshape(P, LNC * 2))
        perm = np.roll(np.arange(E), -e)   # col 0 = own expert
        in_maps.append({
            "xT": xT_t,
            "xg_t": xg_t,
            "gw": np.ascontiguousarray(gate_w[:, perm]),
            "gb": np.ascontiguousarray(
                np.broadcast_to(gate_b[perm], (P, E))),
            "w1": w1t,
            "b1": np.ascontiguousarray(b1[e]),
            "w2": w2t,
            "b2": np.ascontiguousarray(np.broadcast_to(b2[e], (P, D))),
            "gidx": gidx_core,
            "dest2": dest2s[e],
            "ln_g": np.ascontiguousarray(np.broadcast_to(ln_g, (P, D))),
            "ln_b": np.ascontiguousarray(np.broadcast_to(ln_b, (P, D))),
        })

    return C, tuple(C2s), (Bb, S, D, F, E, T), in_maps


def kernel(x, gate_w, gate_b, w1, b1, w2, b2, ln_g, ln_b):
    C, C2s, (Bb, S, D, F, E, T), in_maps = prepare(
        x, gate_w, gate_b, w1, b1, w2, b2, ln_g, ln_b)
    nc = _get_program(C, C2s, D, F, E, T)
    res = bass_utils.run_bass_kernel_spmd(
        nc, in_maps, core_ids=list(range(N_CORES)))
    shards = [res.results[c]["out"] for c in range(N_CORES)]
    return np.concatenate(shards, axis=0).reshape(Bb, S, D)


# revision 61
# speedup vs baseline: 1.0297x; 1.0156x over previous
"""MoE forward (top-2 routed, 8 experts) on 8 Trainium2 NeuronCores.

Strategy: expert-parallel. The host computes the (cheap) routing decisions
and gathers each expert's assigned tokens (padded to capacity C); core e
computes the gating softmax on device (true fp32, so top-2 decisions match
the host routing) and its expert's FFN over its C tokens with fp32r matmuls
(full PE rate, ~1e-4 rounding) accumulated in fp32 PSUM.  Contributions
p * (ffn(x) + b2) are scattered into owner-core-grouped chunks and exchanged
with a single AllToAll; each core then gathers its own tokens' two expert
contributions, sums them, applies LayerNorm, and writes its 512-token shard.
The host concatenates the 8 shards.

Self-contained: hardcodes the problem shapes; compiles per routing capacity.
"""
import numpy as np

import concourse.bass as bass
import concourse.bacc as bacc
import concourse.tile as tile
import concourse.mybir as mybir
import concourse.bass_utils as bass_utils
from concourse.bass import IndirectOffsetOnAxis

P = 128
N_CORES = 8
TEMP = 0.9
EPS = 1e-5

f32 = mybir.dt.float32
f32r = mybir.dt.float32r
i32 = mybir.dt.int32
AF = mybir.ActivationFunctionType
OP = mybir.AluOpType


def _slices(C):
    """Token-slot slices for the compute/AllToAll pipeline (multiples of P).

    Single slice: a 2-slice pipeline was tried and the overlapped AllToAll's
    DMA traffic starved the PE of weight panels (net wash)."""
    return [(0, C)]


def _sub_blocks(n, pref=512):
    """Split n (multiple of 128) into matmul free-dim blocks <= 512,
    preferring >=256 (full-rate fp32r)."""
    out = []
    rem = n
    while rem > 0:
        if rem == 128 + pref:
            out += [384, 256]
            rem = 0
        elif rem >= pref:
            out.append(pref)
            rem -= pref
        else:
            out.append(rem)
            rem = 0
    return out


def build(C, C2s, D=1024, F=4096, E=8, T=4096, ln_triv=False, b2_zero=False):
    """Build the SPMD Bass program for capacity C (multiple of 128).

    C2s[s] = fixed per-(expert, owner-core) group capacity for slice s's
    AllToAll (one collective per compute slice, pipelined)."""
    DC, FC = D // P, F // P
    MG = FC // 4                  # m-groups of 4 F-chunks (w1/w2 panel unit)
    NCk = C // P                  # token chunks per core
    TOK = T // N_CORES            # tokens per output shard
    LNC = TOK // P                # LayerNorm tiles per shard
    assert D % P == 0 and F % (4 * P) == 0 and TOK % P == 0 and C % P == 0

    nc = bacc.Bacc("TRN2", target_bir_lowering=False, debug=False,
                   enable_asserts=True, num_devices=N_CORES)

    # All big inputs are host pre-tiled to [.., P, free] so each DMA is 128
    # contiguous per-partition descriptors (DMA queues are descriptor-bound
    # otherwise).
    xT = nc.dram_tensor("xT", [P, DC * C], f32, kind="ExternalInput").ap()
    xg_t = nc.dram_tensor("xg_t", [NCk, P, DC * P], f32,
                          kind="ExternalInput").ap()
    gw = nc.dram_tensor("gw", [D, E], f32, kind="ExternalInput").ap()
    gb = nc.dram_tensor("gb", [P, E], f32, kind="ExternalInput").ap()
    w1 = nc.dram_tensor("w1", [MG, P, DC * 4 * P], f32r,
                        kind="ExternalInput").ap()
    b1 = nc.dram_tensor("b1", [F], f32, kind="ExternalInput").ap()
    w2 = nc.dram_tensor("w2", [MG, P, 4 * D], f32r,
                        kind="ExternalInput").ap()
    b2 = nc.dram_tensor("b2", [P, D], f32, kind="ExternalInput").ap()
    # [q, ch, 2]: for shard token ch*P+q, the two source rows (e*C2 + pos)
    # in its half's post-AllToAll contribution tensor.
    gidx = nc.dram_tensor("gidx", [P, LNC * 2], i32, kind="ExternalInput").ap()
    # [slot]: destination row in the combined pre-AllToAll tensor (half h's
    # region starts at 8*sum(C2s[:h])); pad slots hold 2^31-1 and are dropped
    # by the scatter's bounds check.
    NH = len(C2s)
    dest2 = nc.dram_tensor("dest2", [C], i32, kind="ExternalInput").ap()
    lng = nc.dram_tensor("ln_g", [P, D], f32, kind="ExternalInput").ap()
    lnb = nc.dram_tensor("ln_b", [P, D], f32, kind="ExternalInput").ap()
    out = nc.dram_tensor("out", [TOK, D], f32, kind="ExternalOutput").ap()

    with tile.TileContext(nc) as tc:
        with (
            tc.tile_pool(name="res", bufs=1) as res,          # resident tiles
            tc.tile_pool(name="wpan", bufs=3) as wpan,        # weight panels
            tc.tile_pool(name="hbuf", bufs=1) as hbuf,        # h (gelu out)
            tc.tile_pool(name="work", bufs=2) as work,        # LN / misc
            tc.tile_pool(name="gat", bufs=1) as gat,          # gating temps
            tc.tile_pool(name="ps1", bufs=3, space="PSUM") as ps1p,
            tc.tile_pool(name="ps2", bufs=4, space="PSUM") as ps2p,
            tc.tile_pool(name="dram", bufs=1, space="DRAM") as dram,
        ):
            # ---- gating inputs first: PE can start while x/consts stream --
            gw_sb = res.tile([P, DC, E], f32)
            nc.sync.dma_start(gw_sb[:], gw.rearrange("(dc p) e -> p dc e", p=P))
            gb_sb = res.tile([P, E], f32)
            nc.sync.dma_start(gb_sb[:], gb[:, :])

            # ---------------- gating (fp32) ----------------
            logit = gat.tile([P, NCk, E], f32)
            for t in range(NCk):
                # True-fp32 load of this token chunk straight from DRAM: the
                # fp32r copy x_sb is rounded, and f32r rounding noise in the
                # gating logits can flip top-2 decisions vs the host routing.
                xg = work.tile([P, DC, P], f32, tag="xg")
                nc.sync.dma_start(
                    xg[:], xg_t[t].rearrange("p (dc q) -> p dc q", dc=DC))
                pg = ps1p.tile([P, E], f32, tag="ps1")
                for dc in range(DC):
                    nc.tensor.matmul(
                        pg[:],
                        xg[:, dc, :],
                        gw_sb[:, dc, :],
                        start=(dc == 0), stop=(dc == DC - 1),
                    )
                nc.vector.tensor_add(logit[:, t, :], pg[:], gb_sb[:])

            # ---------------- resident loads ----------------
            x_sb = res.tile([P, DC, C], f32r)
            # gpsimd cast DMAs: round fp32 -> fp32r once here (PE input prep);
            # split per D-chunk so the loads spread across DMA queues.
            for dc in range(DC):
                nc.gpsimd.dma_start(x_sb[:, dc, :], xT[:, dc * C:(dc + 1) * C])
            b1_sb = res.tile([P, FC], f32)
            nc.sync.dma_start(b1_sb[:], b1.rearrange("(fc p) -> p fc", p=P))
            b2_sb = res.tile([P, D], f32)
            nc.sync.dma_start(b2_sb[:], b2[:, :])
            lng_sb = res.tile([P, D], f32)
            nc.sync.dma_start(lng_sb[:], lng[:, :])
            lnb_sb = res.tile([P, D], f32)
            nc.sync.dma_start(lnb_sb[:], lnb[:, :])
            gidx_sb = res.tile([P, LNC, 2], i32)
            nc.sync.dma_start(gidx_sb[:],
                              gidx.rearrange("p (c k) -> p c k", k=2))
            dest2_sb = res.tile([P, NCk], i32)
            nc.sync.dma_start(dest2_sb[:], dest2.rearrange("(c p) -> p c", p=P))
            eps_sb = res.tile([P, 1], f32)
            nc.vector.memset(eps_sb[:], EPS)

            m1 = gat.tile([P, NCk, 1], f32)
            nc.vector.tensor_reduce(m1[:], logit[:], axis=mybir.AxisListType.X,
                                    op=OP.max)
            tt = gat.tile([P, NCk, E], f32)
            nc.vector.tensor_tensor(tt[:], logit[:],
                                    m1[:].to_broadcast([P, NCk, E]), OP.subtract)
            eq = gat.tile([P, NCk, E], f32)
            nc.vector.tensor_scalar(eq[:], tt[:], 0.0, None, OP.is_equal)
            msk = gat.tile([P, NCk, E], f32)
            nc.vector.scalar_tensor_tensor(msk[:], eq[:], -1e30, tt[:],
                                           OP.mult, OP.add)
            m2 = gat.tile([P, NCk, 1], f32)
            nc.vector.tensor_reduce(m2[:], msk[:], axis=mybir.AxisListType.X,
                                    op=OP.max)
            keep = gat.tile([P, NCk, E], f32)
            nc.vector.tensor_tensor(keep[:], tt[:],
                                    m2[:].to_broadcast([P, NCk, E]), OP.is_ge)
            ex = gat.tile([P, NCk, E], f32)
            nc.scalar.activation(ex[:], tt[:], AF.Exp, scale=float(1.0 / TEMP))
            exk = gat.tile([P, NCk, E], f32)
            nc.vector.tensor_mul(exk[:], ex[:], keep[:])
            ssum = gat.tile([P, NCk, 1], f32)
            nc.vector.tensor_reduce(ssum[:], exk[:], axis=mybir.AxisListType.X,
                                    op=OP.add)
            rcp = gat.tile([P, NCk, 1], f32)
            nc.vector.reciprocal(rcp[:], ssum[:])
            pcols = gat.tile([P, NCk], f32)
            nc.vector.tensor_tensor(pcols[:], exk[:, :, 0],
                                    rcp[:, :, 0], OP.mult)

            # ---------------- y accumulator init: y = p * b2 --------------
            y_acc = res.tile([P, NCk, D], f32)
            if b2_zero:
                nc.gpsimd.memset(y_acc[:], 0.0)
            else:
                for mt in range(NCk):
                    nc.vector.tensor_scalar(y_acc[:, mt, :], b2_sb[:],
                                            pcols[:, mt:mt + 1], None, OP.mult)

            # ---------------- main FFN loop ----------------
            ndsubs = _sub_blocks(D)
            for sc0, scn in _slices(C):
                for mg in range(MG):
                    w1p = wpan.tile([P, DC, 4 * P], f32r, tag="wp")
                    nc.sync.dma_start(
                        w1p[:], w1[mg].rearrange("p (dc m) -> p dc m", dc=DC))
                    w2p = wpan.tile([P, 4, D], f32r, tag="wp")
                    nc.sync.dma_start(
                        w2p[:], w2[mg].rearrange("p (mi d) -> p mi d", mi=4))
                    h_mg = hbuf.tile([P, 4, scn], f32r, tag="h")
                    # mm1: h = gelu(x @ w1 + b1), transposed [F-part, tokens]
                    for mi in range(4):
                        c0 = 0
                        for ns in _sub_blocks(scn):
                            ps = ps1p.tile([P, ns], f32, tag="ps1")
                            for dc in range(DC):
                                nc.tensor.matmul(
                                    ps[:],
                                    w1p[:, dc, mi * P:(mi + 1) * P],
                                    x_sb[:, dc, sc0 + c0:sc0 + c0 + ns],
                                    start=(dc == 0), stop=(dc == DC - 1),
                                )
                            nc.scalar.activation(
                                h_mg[:, mi, c0:c0 + ns], ps[:], AF.Gelu,
                                bias=b1_sb[:, mg * 4 + mi:mg * 4 + mi + 1])
                            c0 += ns
                    # mm2: y[tok, d] += p * (h.T @ w2)
                    for mtl in range(scn // P):
                        mt = sc0 // P + mtl
                        d0 = 0
                        for nd in ndsubs:
                            ps = ps2p.tile([P, nd], f32, tag="ps2")
                            for mi in range(4):
                                nc.tensor.matmul(
                                    ps[:],
                                    h_mg[:, mi, mtl * P:(mtl + 1) * P],
                                    w2p[:, mi, d0:d0 + nd],
                                    start=(mi == 0), stop=(mi == 3),
                                )
                            nc.vector.scalar_tensor_tensor(
                                y_acc[:, mt, d0:d0 + nd], ps[:],
                                pcols[:, mt:mt + 1], y_acc[:, mt, d0:d0 + nd],
                                OP.mult, OP.add)
                            d0 += nd

            # ---- ship contributions in NH owner-position halves: the LN of
            # half h overlaps half h+1's AllToAll transfer.  One scatter per
            # chunk into a combined tensor; each half's region feeds its own
            # collective.
            ya_off = [0]
            for h in range(NH):
                ya_off.append(ya_off[-1] + N_CORES * C2s[h])
            y_all = dram.tile([ya_off[-1], D], f32)
            for mt in range(NCk):
                nc.gpsimd.indirect_dma_start(
                    out=y_all[:, :],
                    out_offset=IndirectOffsetOnAxis(
                        ap=dest2_sb[:, mt:mt + 1], axis=0),
                    in_=y_acc[:, mt, :],
                    in_offset=None,
                    bounds_check=ya_off[-1] - 1,
                    oob_is_err=False,
                )
            ags = []
            for h in range(NH):
                ag = dram.tile([N_CORES * C2s[h], D], f32, tag=f"ag{h}",
                               name=f"ag{h}")
                nc.gpsimd.collective_compute(
                    "AllToAll",
                    OP.bypass,
                    replica_groups=[list(range(N_CORES))],
                    ins=[y_all[ya_off[h]:ya_off[h + 1], :].opt()],
                    outs=[ag.opt()],
                )
                ags.append(ag)

            # -------- per-shard combine (2 contributions) + LayerNorm ------
            for ch in range(LNC):
                ag = ags[ch * NH // LNC]
                xt = work.tile([P, D], f32, tag="ln_x")
                nc.gpsimd.indirect_dma_start(
                    out=xt[:], out_offset=None,
                    in_=ag[:, :],
                    in_offset=IndirectOffsetOnAxis(
                        ap=gidx_sb[:, ch, 0:1], axis=0),
                )
                # second gather accumulates in place (DMA compute-op add)
                nc.gpsimd.indirect_dma_start(
                    out=xt[:], out_offset=None,
                    in_=ag[:, :],
                    in_offset=IndirectOffsetOnAxis(
                        ap=gidx_sb[:, ch, 1:2], axis=0),
                    compute_op=OP.add,
                )
                s1 = work.tile([P, 1], f32, tag="ln_s1")
                nc.vector.tensor_reduce(s1[:], xt[:], axis=mybir.AxisListType.X,
                                        op=OP.add)
                scr = work.tile([P, D], f32, tag="ln_scr")
                s2 = work.tile([P, 1], f32, tag="ln_s2")
                nc.scalar.activation(scr[:], xt[:], AF.Square, accum_out=s2[:])
                mean = work.tile([P, 1], f32, tag="ln_mean")
                nc.vector.tensor_scalar_mul(mean[:], s1[:], 1.0 / D)
                ex2 = work.tile([P, 1], f32, tag="ln_ex2")
                nc.vector.tensor_scalar_mul(ex2[:], s2[:], 1.0 / D)
                # var = ex2 - mean^2  (computed as (-mean)*mean + ex2)
                var = work.tile([P, 1], f32, tag="ln_var")
                nc.vector.scalar_tensor_tensor(var[:], mean[:], -1.0, mean[:],
                                               OP.mult, OP.mult)
                nc.vector.tensor_add(var[:], var[:], ex2[:])
                sd = work.tile([P, 1], f32, tag="ln_sd")
                nc.scalar.activation(sd[:], var[:], AF.Sqrt, bias=eps_sb[:])
                rstd = work.tile([P, 1], f32, tag="ln_rstd")
                nc.vector.reciprocal(rstd[:], sd[:])
                nrm = work.tile([P, D], f32, tag="ln_nrm")
                nc.vector.tensor_scalar(nrm[:], xt[:], mean[:], rstd[:],
                                        OP.subtract, OP.mult)
                if not ln_triv:
                    nc.vector.tensor_mul(nrm[:], nrm[:], lng_sb[:])
                    nc.vector.tensor_add(nrm[:], nrm[:], lnb_sb[:])
                nc.sync.dma_start(out[ch * P:(ch + 1) * P, :], nrm[:])

    nc.compile()
    return nc


_CACHE = {}


def _get_program(C, C2s, D, F, E, T, ln_triv, b2_zero):
    key = (C, tuple(C2s), D, F, E, T, ln_triv, b2_zero)
    if key not in _CACHE:
        _CACHE[key] = build(C, tuple(C2s), D=D, F=F, E=E, T=T,
                            ln_triv=ln_triv, b2_zero=b2_zero)
    return _CACHE[key]


def prepare(x, gate_w, gate_b, w1, b1, w2, b2, ln_g, ln_b):
    """Host-side routing + per-core input construction.

    Returns (C, dims, in_maps)."""
    x = np.asarray(x, np.float32)
    gate_w = np.asarray(gate_w, np.float32)
    gate_b = np.asarray(gate_b, np.float32)
    w1 = np.asarray(w1, np.float32)
    b1 = np.asarray(b1, np.float32)
    w2 = np.asarray(w2, np.float32)
    b2 = np.asarray(b2, np.float32)
    ln_g = np.asarray(ln_g, np.float32)
    ln_b = np.asarray(ln_b, np.float32)

    Bb, S, D = x.shape
    E, _, F = w1.shape
    T = Bb * S
    TOK = T // N_CORES

    # ---- host routing (decisions only; probabilities are computed on device)
    x2 = np.ascontiguousarray(x.reshape(T, D))
    logits = (x2 @ gate_w + gate_b) / TEMP
    top2 = np.argsort(-logits, axis=1, kind="stable")[:, :2]
    idx = [np.where((top2 == e).any(axis=1))[0] for e in range(E)]
    maxc = max(len(i) for i in idx)
    C = max(((maxc + P - 1) // P) * P, P)

    DC, NCk, MG = D // P, C // P, F // (4 * P)
    LNC = TOK // P

    # Owner-(core, shard-half) grouping for the AllToAlls: half h carries the
    # contributions of shard rows [h*TOK/NH, (h+1)*TOK/NH) of every core, so
    # half 0's LayerNorm overlaps half 1's transfer.  Within an expert batch
    # (ids ascending) each (owner, half) group is a contiguous run.
    NH = 1  # half-split A2A measured as a wash vs single collective
    HTOK = TOK // NH
    cnt_h = np.zeros((NH, E, N_CORES), np.int64)
    for e in range(E):
        ids = idx[e]
        owner = ids // TOK
        half = (ids % TOK) // HTOK
        for h in range(NH):
            cnt_h[h, e] = np.bincount(owner[half == h], minlength=N_CORES)
    C2s = [max(16, int(((cnt_h[h].max() + 15) // 16) * 16))
           for h in range(NH)]

    # rows[t, k]: source row of (token t, choice k) in its half's
    # post-AllToAll tensor on t's owner core.  Pads / other-half slots get
    # BIG and are dropped by the scatter bounds check.
    BIG = np.int32(2**31 - 1)
    ya_off = np.concatenate(
        [[0], np.cumsum([N_CORES * c2 for c2 in C2s])]).astype(np.int64)
    rows = np.empty((T, 2), np.int32)
    dest2s = [np.full((C,), BIG, np.int32) for _ in range(E)]
    for e in range(E):
        ids = idx[e]
        n = len(ids)
        owner = ids // TOK
        half = (ids % TOK) // HTOK
        key = half * N_CORES + owner
        kcnt = np.bincount(key, minlength=NH * N_CORES)
        gstart = np.concatenate([[0], np.cumsum(kcnt)[:-1]])
        order = np.argsort(key, kind="stable")
        pos = np.empty(n, np.int64)
        pos[order] = np.arange(n) - np.repeat(gstart, kcnt)
        dest2s[e][:n] = (ya_off[half] + owner * np.take(C2s, half)
                         + pos).astype(np.int32)
        for k in (0, 1):
            mk = top2[ids, k] == e
            rows[ids[mk], k] = (
                e * np.take(C2s, half[mk]) + pos[mk]).astype(np.int32)

    in_maps = []
    for e in range(E):
        ids = idx[e]
        xTe = np.zeros((D, C), np.float32)
        xTe[:, :len(ids)] = x2[ids].T
        xT_t = np.ascontiguousarray(
            xTe.reshape(DC, P, C).transpose(1, 0, 2).reshape(P, DC * C))
        xg_t = np.ascontiguousarray(
            xTe.reshape(DC, P, NCk, P).transpose(2, 1, 0, 3)
            .reshape(NCk, P, DC * P))
        w1t = np.ascontiguousarray(
            w1[e].reshape(DC, P, MG, 4 * P).transpose(2, 1, 0, 3)
            .reshape(MG, P, DC * 4 * P))
        w2t = np.ascontiguousarray(
            w2[e].reshape(MG, 4, P, D).transpose(0, 2, 1, 3)
            .reshape(MG, P, 4 * D))
        gidx_core = np.ascontiguousarray(
            rows[e * TOK:(e + 1) * TOK].reshape(LNC, P, 2)
            .transpose(1, 0, 2).reshape(P, LNC * 2))
        perm = np.roll(np.arange(E), -e)   # col 0 = own expert
        in_maps.append({
            "xT": xT_t,
            "xg_t": xg_t,
            "gw": np.ascontiguousarray(gate_w[:, perm]),
            "gb": np.ascontiguousarray(
                np.broadcast_to(gate_b[perm], (P, E))),
            "w1": w1t,
            "b1": np.ascontiguousarray(b1[e]),
            "w2": w2t,
            "b2": np.ascontiguousarray(np.broadcast_to(b2[e], (P, D))),
            "gidx": gidx_core,
            "dest2": dest2s[e],
            "ln_g": np.ascontiguousarray(np.broadcast_to(ln_g, (P, D))),
            "ln_b": np.ascontiguousarray(np.broadcast_to(ln_b, (P, D))),
        })

    ln_triv = bool(np.all(ln_g == 1.0) and np.all(ln_b == 0.0))
    b2_zero = bool(np.all(b2 == 0.0))
    return C, tuple(C2s), (Bb, S, D, F, E, T), (ln_triv, b2_zero), in_maps


def kernel(x, gate_w, gate_b, w1, b1, w2, b2, ln_g, ln_b):
    C, C2s, (Bb, S, D, F, E, T), (ln_triv, b2_zero), in_maps = prepare(
        x, gate_w, gate_b, w1, b1, w2, b2, ln_g, ln_b)
    nc = _get_program(C, C2s, D, F, E, T, ln_triv, b2_zero)
    res = bass_utils.run_bass_kernel_spmd(
        nc, in_maps, core_ids=list(range(N_CORES)))
    shards = [res.results[c]["out"] for c in range(N_CORES)]
    return np.concatenate(shards, axis=0).reshape(Bb, S, D)
